# revision 1
# baseline (speedup 1.0000x reference)
"""Unfold/im2col kernel for Trainium2 (Bass/Tile), 8-core data parallel.

Problem: x [4, 64, 224, 224] f32 -> out [4, 576, 49729] f32 where
out[b, (c*3+kh)*3+kw, oh*223+ow] = pad(x,1)[b, c, oh+kh, ow+kw]
(3x3 kernel, pad 1, stride 1, dilation 1, oh=ow=223).

Sharding: 8 cores = (batch 4) x (channel half 2). Each core handles
32 channels -> [288, 49729] independently; outputs concatenate on the
channel axis (channel-major row layout makes halves contiguous).

The input is zero-padded host-side to [32, 226, 226] per core, so the
device kernel is pure DMA. All 32 padded images live in two SBUF tiles
(padded rows 0..127 / 128..225 on partitions, channels side by side in
the free dim), each filled by ONE load DMA. Each (kh, kw) window is
then written by one DMA per tile half per 16-channel block via a 3D
access pattern (window-row x channel x 223). Big stores issue on
gpsimd (SWDGE): its model-queue DMAs are spread across all 16 SDMA
engines (~230 GB/s at this 892 B descriptor size), whereas the HWDGE
dynamic rings feed a single SDMA engine (~15-28 GB/s) and only carry
the tiny split-remainder chunks. Measured ~308 us/core on TRN2
(roofline for 57 MB out + 6.5 MB in at ~358 GB/s HBM is ~180 us; the
892 B descriptor processing rate of the SDMA engines is the binding
limit).
"""

from contextlib import ExitStack

import numpy as np

import concourse.bass as bass
import concourse.tile as tile
from concourse import mybir
from concourse.ap import AP
from concourse.bass_utils import run_bass_kernel_spmd

B, C, IH, IW = 4, 64, 224, 224
N_CORES = 8
CPC = C // 2          # channels per core: 32
PH = IH + 2           # padded height/width: 226
OH = IH - 1           # output spatial: 223
OSZ = OH * OH         # 49729
NROW = CPC * 9        # 288 output rows per core
ROWS0 = 128           # padded rows 0..127 in tile0
ROWS1 = PH - ROWS0    # padded rows 128..225 in tile1 (98)
FREE = CPC * PH       # free dim elements per tile: 7232
PIMG = PH * PH        # padded image elements: 51076

_NC_CACHE = {}


def build_nc() -> bass.Bass:
    nc = bass.Bass()
    x = nc.declare_dram_parameter("xp", [CPC, PH, PH], mybir.dt.float32, isOutput=False)
    out = nc.declare_dram_parameter("out", [NROW, OSZ], mybir.dt.float32, isOutput=True)
    xb = x[:, :, :]
    ob = out[:, :]

    with tile.TileContext(nc) as tc:
        with ExitStack() as ctx:
            pool = ctx.enter_context(tc.tile_pool(name="img", bufs=1))
            t0 = pool.tile([ROWS0, FREE], mybir.dt.float32, name="t0", tag="t0")[:, :]
            t1 = pool.tile([ROWS1, FREE], mybir.dt.float32, name="t1", tag="t1")[:, :]

            # Two loads: tile partition p, free col c*226+w  <-  xp[c, p(+128), w]
            src0 = AP(xb.tensor, xb.offset,
                      [[PH, ROWS0], [PIMG, CPC], [1, PH]])
            dst0 = AP(t0.tensor, t0.offset,
                      [[FREE, ROWS0], [PH, CPC], [1, PH]])
            nc.gpsimd.dma_start(out=dst0, in_=src0)
            src1 = AP(xb.tensor, xb.offset + ROWS0 * PH,
                      [[PH, ROWS1], [PIMG, CPC], [1, PH]])
            dst1 = AP(t1.tensor, t1.offset,
                      [[FREE, ROWS1], [PH, CPC], [1, PH]])
            nc.gpsimd.dma_start(out=dst1, in_=src1)

            # Stores: for each (kh, kw), 16 channels per DMA (the channel
            # dim is split in half so the (window-row, channel, col) walk
            # keeps the partition-crossing step on dim 0 and no dim merge
            # fires; 32-channel and 4-channel variants measured slower).
            # out row (c*9 + kh*3 + kw), col r*223.. = padded[kh+r, kw..kw+222];
            # window rows 0..n0-1 live in tile0 (partitions kh..127), the rest
            # in tile1 (partitions 0..n1-1).
            # Row counts 97/113/127 crash the SWDGE path on device
            # (NRT_EXEC_UNIT_UNRECOVERABLE, found empirically), so split
            # those transfers into known-good chunk sizes.
            def safe_rows(n):
                if n in (128, 126, 124, 121, 120, 112, 96, 95, 64, 63, 31, 15, 1):
                    return [n]
                for first in (112, 96, 64):
                    if 0 < n - first and (n - first) in (63, 31, 15, 1):
                        return [first, n - first]
                return [n - 15, 15]

            # Each store: (kh, kw, h, tile, chunk-start-row r, rows n).
            CH2 = CPC // 2
            work = []
            for kh in range(3):
                n0 = ROWS0 - kh
                n1 = OH - n0
                for kw in range(3):
                    for h in range(2):
                        r = 0
                        for n in safe_rows(n0):
                            work.append((kh, kw, h, 0, r, n))
                            r += n
                        for n in safe_rows(n1):
                            work.append((kh, kw, h, 1, r, n))
                            r += n

            def emit(eng, kh, kw, h, tl, r, n):
                co = h * CH2
                if tl == 0:
                    src = AP(t0.tensor,
                             t0.offset + (kh + r) * FREE + co * PH + kw,
                             [[FREE, n], [PH, CH2], [1, OH]])
                else:
                    src = AP(t1.tensor,
                             t1.offset + (r - (ROWS0 - kh)) * FREE + co * PH + kw,
                             [[FREE, n], [PH, CH2], [1, OH]])
                dst = AP(ob.tensor,
                         ob.offset + (co * 9 + kh * 3 + kw) * OSZ + r * OH,
                         [[OH, n], [9 * OSZ, CH2], [1, OH]])
                eng.dma_start(out=dst, in_=src)

            # Tiny split-remainder chunks go to the (otherwise idle) HWDGE
            # queues; the big stores stay on the fast SWDGE model queue,
            # ordered tile0-first so the queue never stalls on load1.
            small = [w for w in work if w[5] <= 15]
            big = [w for w in work if w[5] > 15]
            for i, (kh, kw, h, tl, r, n) in enumerate(small):
                emit(nc.sync if i % 2 == 0 else nc.scalar, kh, kw, h, tl, r, n)
            for kh, kw, h, tl, r, n in sorted(big, key=lambda w: w[3]):
                emit(nc.gpsimd, kh, kw, h, tl, r, n)
    return nc


def _split_multi_waits(nc: bass.Bass) -> None:
    """Walrus allows only one sync-wait command per instruction (the
    kernel-tail drain ends up with one per DMA-completion sem lane).
    Hoist all but the last wait onto fresh single-wait NOPs inserted
    just before the instruction on the same engine — semantically
    identical (the engine blocks on each wait in turn)."""
    from bass_rust import SyncInfo

    k = 0
    for fn in nc.m.functions:
        for blk in fn.blocks:
            insts = blk.instructions
            for idx in range(len(insts) - 1, -1, -1):
                inst = insts[idx]
                si = inst.sync_info
                if si is None or len(si.on_wait) <= 1:
                    continue
                waits = list(si.on_wait)
                for w in waits[:-1]:
                    nop = mybir.InstNoOp(name=f"WSPLIT-{k}")
                    k += 1
                    nop.engine = inst.engine
                    nop.sync_info = SyncInfo(on_wait=[w], on_update=[])
                    insts.insert(idx, nop)
                si.on_wait = [waits[-1]]
                inst.sync_info = si


def get_nc() -> bass.Bass:
    if "nc" not in _NC_CACHE:
        nc = build_nc()
        _split_multi_waits(nc)
        _NC_CACHE["nc"] = nc
    return _NC_CACHE["nc"]


def make_in_maps(x: np.ndarray) -> list[dict]:
    x = np.asarray(x, dtype=np.float32)
    xp = np.pad(x, ((0, 0), (0, 0), (1, 1), (1, 1)))
    maps = []
    for core in range(N_CORES):
        b, half = divmod(core, 2)
        maps.append({"xp": np.ascontiguousarray(xp[b, half * CPC:(half + 1) * CPC])})
    return maps


def gather_out(results: list[dict]) -> np.ndarray:
    out = np.empty((B, C * 9, OSZ), dtype=np.float32)
    for core in range(N_CORES):
        b, half = divmod(core, 2)
        out[b, half * NROW:(half + 1) * NROW] = results[core]["out"]
    return out


def kernel(**inputs) -> np.ndarray:
    x = inputs["x"]
    nc = get_nc()
    res = run_bass_kernel_spmd(nc, make_in_maps(x), list(range(N_CORES)))
    return gather_out(res.results)



# revision 3
# speedup vs baseline: 1.4005x; 1.4005x over previous
"""Unfold/im2col kernel for Trainium2 (Bass/Tile), 8-core data parallel.

Problem: x [4, 64, 224, 224] f32 -> out [4, 576, 49729] f32 where
out[b, (c*3+kh)*3+kw, oh*223+ow] = pad(x,1)[b, c, oh+kh, ow+kw]
(3x3 kernel, pad 1, stride 1, dilation 1, oh=ow=223).

Sharding: 8 cores = (batch 4) x (channel half 2). Each core handles
32 channels -> [288, 49729] independently; outputs concatenate on the
channel axis (channel-major row layout makes halves contiguous).

Per-core strategy (v3 — big-descriptor stores, half-window pipeline):
The input is zero-padded host-side to [32, 226, 226]. SBUF partition
p = g*32 + c holds consecutive padded rows of channel c (row-group g
covers output rows R0[g]..R0[g]+55, split into two 28-row halves with
a 2-row halo between the half tiles), loaded with one contiguous
~27 KB descriptor per partition. For each of the 9 (kh, kw) windows,
DVE copies compact the 226-wide padded rows into the output's exact
DRAM layout (28 dense rows of 223) in half-window tiles (2 windows
double-buffered); each store DMA then moves ~25 KB fully-contiguous
runs per partition. This replaces the v1 pure-DMA scheme whose 892 B
descriptors bound the SDMA engines at ~230 GB/s; big descriptors run
at the HBM per-core roofline (~358 GB/s). Loads are issued on the ACT
HWDGE ring so their descriptors interleave with store descriptors
(SP ring) at the SDMA engines, letting the first stores overlap the
tail of the load phase; half-window granularity shortens the
load->copy->store serial head and the WAR wait for compacted-buffer
reuse. Group store order g0,g2,g1,g3 alternates the even-engine
(partitions 0-63) and odd-engine (64-127) SDMA halves.
"""

from contextlib import ExitStack

import numpy as np

import concourse.bass as bass
import concourse.tile as tile
from concourse import mybir
from concourse.ap import AP
from concourse.bass_utils import run_bass_kernel_spmd

B, C, IH, IW = 4, 64, 224, 224
N_CORES = 8
CPC = C // 2          # channels per core: 32
PH = IH + 2           # padded height/width: 226
OH = IH - 1           # output spatial: 223
OSZ = OH * OH         # 49729
NROW = CPC * 9        # 288 output rows per core
PIMG = PH * PH        # padded image elements: 51076

R0 = [0, 56, 112, 168]        # first output row of each group
IMGH_ROWS = 30                # padded rows per partition per half tile
IMGH_F = IMGH_ROWS * PH       # img half-tile free size: 6780
CBH_ROWS = 28                 # compacted rows per half (g3 h1 row 27 is junk)
CBH_F = CBH_ROWS * OH         # compacted half-tile free size: 6244

_NC_CACHE = {}


def build_nc() -> bass.Bass:
    nc = bass.Bass()
    x = nc.declare_dram_parameter("xp", [CPC, PH, PH], mybir.dt.float32, isOutput=False)
    out = nc.declare_dram_parameter("out", [NROW, OSZ], mybir.dt.float32, isOutput=True)
    xb = x[:, :, :]
    ob = out[:, :]

    # interleave even-engine (g0,g1 -> partitions 0..63) and odd-engine
    # (g2,g3 -> partitions 64..127) groups so all 16 SDMA engines stay fed
    GORDER = (0, 2, 1, 3)

    with tile.TileContext(nc) as tc:
        with ExitStack() as ctx:
            pool = ctx.enter_context(tc.tile_pool(name="img", bufs=1))
            imgh = [
                pool.tile([128, IMGH_F], mybir.dt.float32, name=f"img{h}", tag=f"img{h}")[:, :]
                for h in range(2)
            ]
            # cb[j][h]: half h of window-buffer j (windows double-buffered)
            cb = [
                [
                    pool.tile([128, CBH_F], mybir.dt.float32,
                              name=f"cb{j}{h}", tag=f"cb{j}{h}")[:, :]
                    for h in range(2)
                ]
                for j in range(2)
            ]

            # Loads (ACT HWDGE ring): partition g*32+c of img half h holds
            # padded rows R0[g]+28h .. +30 of channel c (one 27120 B
            # contiguous descriptor per partition).
            for h in range(2):
                for g in GORDER:
                    src = AP(xb.tensor, xb.offset + (R0[g] + 28 * h) * PH,
                             [[PIMG, CPC], [1, IMGH_F]])
                    dst = AP(imgh[h].tensor, imgh[h].offset + (g * CPC) * IMGH_F,
                             [[IMGH_F, CPC], [1, IMGH_F]])
                    nc.scalar.dma_start(out=dst, in_=src)

            # Per window (kh, kw) and half h: DVE compacts 28 dense rows of
            # 223, then 4 store DMAs (SP ring) move ~25 KB contiguous runs.
            for w in range(9):
                kh, kw = divmod(w, 3)
                for h in range(2):
                    c_src = AP(imgh[h].tensor, imgh[h].offset + kh * PH + kw,
                               [[IMGH_F, 128], [PH, CBH_ROWS], [1, OH]])
                    c_dst = AP(cb[w % 2][h].tensor, cb[w % 2][h].offset,
                               [[CBH_F, 128], [OH, CBH_ROWS], [1, OH]])
                    nc.vector.tensor_copy(out=c_dst, in_=c_src)

                    for g in GORDER:
                        n = 27 if (g == 3 and h == 1) else 28
                        s = AP(cb[w % 2][h].tensor,
                               cb[w % 2][h].offset + (g * CPC) * CBH_F,
                               [[CBH_F, CPC], [1, n * OH]])
                        d = AP(ob.tensor,
                               ob.offset + (kh * 3 + kw) * OSZ + (R0[g] + 28 * h) * OH,
                               [[9 * OSZ, CPC], [1, n * OH]])
                        nc.sync.dma_start(out=d, in_=s)
    return nc


def _split_multi_waits(nc: bass.Bass) -> None:
    """Walrus allows only one sync-wait command per instruction (the
    kernel-tail drain ends up with one per DMA-completion sem lane).
    Hoist all but the last wait onto fresh single-wait NOPs inserted
    just before the instruction on the same engine — semantically
    identical (the engine blocks on each wait in turn)."""
    from bass_rust import SyncInfo

    k = 0
    for fn in nc.m.functions:
        for blk in fn.blocks:
            insts = blk.instructions
            for idx in range(len(insts) - 1, -1, -1):
                inst = insts[idx]
                si = inst.sync_info
                if si is None or len(si.on_wait) <= 1:
                    continue
                waits = list(si.on_wait)
                for w in waits[:-1]:
                    nop = mybir.InstNoOp(name=f"WSPLIT-{k}")
                    k += 1
                    nop.engine = inst.engine
                    nop.sync_info = SyncInfo(on_wait=[w], on_update=[])
                    insts.insert(idx, nop)
                si.on_wait = [waits[-1]]
                inst.sync_info = si


def get_nc() -> bass.Bass:
    if "nc" not in _NC_CACHE:
        nc = build_nc()
        _split_multi_waits(nc)
        _NC_CACHE["nc"] = nc
    return _NC_CACHE["nc"]


def make_in_maps(x: np.ndarray) -> list[dict]:
    x = np.asarray(x, dtype=np.float32)
    xp = np.pad(x, ((0, 0), (0, 0), (1, 1), (1, 1)))
    maps = []
    for core in range(N_CORES):
        b, half = divmod(core, 2)
        maps.append({"xp": np.ascontiguousarray(xp[b, half * CPC:(half + 1) * CPC])})
    return maps


def gather_out(results: list[dict]) -> np.ndarray:
    out = np.empty((B, C * 9, OSZ), dtype=np.float32)
    for core in range(N_CORES):
        b, half = divmod(core, 2)
        out[b, half * NROW:(half + 1) * NROW] = results[core]["out"]
    return out


def kernel(**inputs) -> np.ndarray:
    x = inputs["x"]
    nc = get_nc()
    res = run_bass_kernel_spmd(nc, make_in_maps(x), list(range(N_CORES)))
    return gather_out(res.results)


# revision 7
# speedup vs baseline: 1.5281x; 1.0911x over previous
"""Unfold/im2col kernel for Trainium2 (Bass/Tile), 8-core data parallel.

Problem: x [4, 64, 224, 224] f32 -> out [4, 576, 49729] f32 where
out[b, (c*3+kh)*3+kw, oh*223+ow] = pad(x,1)[b, c, oh+kh, ow+kw]
(3x3 kernel, pad 1, stride 1, dilation 1, oh=ow=223).

Sharding: 8 cores = (batch 4) x (channel half 2). Each core handles
32 channels -> [288, 49729] independently; outputs concatenate on the
channel axis (channel-major row layout makes halves contiguous).

Per-core strategy (v4 — big-descriptor stores, half-window pipeline):
The input is zero-padded AND pre-sharded host-side into the exact
SBUF layout [128, 13560]: partition p = g*32 + c holds two 30-row
halves of padded rows of channel c (row-group g covers output rows
R0[g]..R0[g]+55, split into two 28-row halves with a 2-row halo
between the half tiles). Each of the two loads is then a full
128-partition DMA with one contiguous ~27 KB descriptor per
partition — keeping every SDMA engine on its own SBUF AXI port
(32-partition loads measured ~11 GB/s/engine from write-port
contention vs ~27 full-rate). For each of the 9 (kh, kw) windows,
DVE copies compact the 226-wide padded rows into the output's exact
DRAM layout (28 dense rows of 223) in half-window tiles (2 windows
double-buffered); each store DMA then moves ~25 KB fully-contiguous
runs per partition. This replaces the v1 pure-DMA scheme whose 892 B
descriptors bound the SDMA engines at ~230 GB/s; big descriptors run
at the HBM per-core roofline (~358 GB/s). Loads are issued on the ACT
HWDGE ring so their descriptors interleave with store descriptors
(SP ring) at the SDMA engines, letting the first stores overlap the
tail of the load phase; half-window granularity shortens the
load->copy->store serial head and the WAR wait for compacted-buffer
reuse. Group store order g0,g2,g1,g3 alternates the even-engine
(partitions 0-63) and odd-engine (64-127) SDMA halves.
"""

from contextlib import ExitStack

import numpy as np

import concourse.bass as bass
import concourse.tile as tile
from concourse import mybir
from concourse.ap import AP
from concourse.bass_utils import run_bass_kernel_spmd

B, C, IH, IW = 4, 64, 224, 224
N_CORES = 8
CPC = C // 2          # channels per core: 32
PH = IH + 2           # padded height/width: 226
OH = IH - 1           # output spatial: 223
OSZ = OH * OH         # 49729
NROW = CPC * 9        # 288 output rows per core
PIMG = PH * PH        # padded image elements: 51076

R0 = [0, 56, 112, 168]        # first output row of each group
IMGH_ROWS = 30                # padded rows per partition per half tile
IMGH_F = IMGH_ROWS * PH       # img half-tile free size: 6780
CBH_ROWS = 28                 # compacted rows per half (g3 h1 row 27 is junk)
CBH_F = CBH_ROWS * OH         # compacted half-tile free size: 6244

_NC_CACHE = {}


def build_nc() -> bass.Bass:
    nc = bass.Bass()
    x = nc.declare_dram_parameter("xp", [128, 2 * IMGH_F], mybir.dt.float32, isOutput=False)
    out = nc.declare_dram_parameter("out", [NROW, OSZ], mybir.dt.float32, isOutput=True)
    xb = x[:, :]
    ob = out[:, :]

    # interleave even-engine (g0,g1 -> partitions 0..63) and odd-engine
    # (g2,g3 -> partitions 64..127) groups so all 16 SDMA engines stay fed
    GORDER = (0, 2, 1, 3)

    with tile.TileContext(nc) as tc:
        with ExitStack() as ctx:
            pool = ctx.enter_context(tc.tile_pool(name="img", bufs=1))
            imgh = [
                pool.tile([128, IMGH_F], mybir.dt.float32, name=f"img{h}", tag=f"img{h}")[:, :]
                for h in range(2)
            ]
            # cb[j][h]: half h of window-buffer j (windows double-buffered)
            cb = [
                [
                    pool.tile([128, CBH_F], mybir.dt.float32,
                              name=f"cb{j}{h}", tag=f"cb{j}{h}")[:, :]
                    for h in range(2)
                ]
                for j in range(2)
            ]

            # Loads (ACT HWDGE ring): the host pre-shards xp so that DRAM
            # row p is partition p's data; each load is one 128-partition
            # DMA with a 27120 B contiguous descriptor per partition.
            for h in range(2):
                src = AP(xb.tensor, xb.offset + h * IMGH_F,
                         [[2 * IMGH_F, 128], [1, IMGH_F]])
                dst = AP(imgh[h].tensor, imgh[h].offset,
                         [[IMGH_F, 128], [1, IMGH_F]])
                nc.scalar.dma_start(out=dst, in_=src)

            # Per window (kh, kw) and half h: DVE compacts 28 dense rows of
            # 223, then 4 store DMAs (SP ring) move ~25 KB contiguous runs.
            for w in range(9):
                kh, kw = divmod(w, 3)
                for h in range(2):
                    c_src = AP(imgh[h].tensor, imgh[h].offset + kh * PH + kw,
                               [[IMGH_F, 128], [PH, CBH_ROWS], [1, OH]])
                    c_dst = AP(cb[w % 2][h].tensor, cb[w % 2][h].offset,
                               [[CBH_F, 128], [OH, CBH_ROWS], [1, OH]])
                    nc.vector.tensor_copy(out=c_dst, in_=c_src)

                    for g in GORDER:
                        n = 27 if (g == 3 and h == 1) else 28
                        s = AP(cb[w % 2][h].tensor,
                               cb[w % 2][h].offset + (g * CPC) * CBH_F,
                               [[CBH_F, CPC], [1, n * OH]])
                        d = AP(ob.tensor,
                               ob.offset + (kh * 3 + kw) * OSZ + (R0[g] + 28 * h) * OH,
                               [[9 * OSZ, CPC], [1, n * OH]])
                        nc.sync.dma_start(out=d, in_=s)
    return nc


def _split_multi_waits(nc: bass.Bass) -> None:
    """Walrus allows only one sync-wait command per instruction (the
    kernel-tail drain ends up with one per DMA-completion sem lane).
    Hoist all but the last wait onto fresh single-wait NOPs inserted
    just before the instruction on the same engine — semantically
    identical (the engine blocks on each wait in turn)."""
    from bass_rust import SyncInfo

    k = 0
    for fn in nc.m.functions:
        for blk in fn.blocks:
            insts = blk.instructions
            for idx in range(len(insts) - 1, -1, -1):
                inst = insts[idx]
                si = inst.sync_info
                if si is None or len(si.on_wait) <= 1:
                    continue
                waits = list(si.on_wait)
                for w in waits[:-1]:
                    nop = mybir.InstNoOp(name=f"WSPLIT-{k}")
                    k += 1
                    nop.engine = inst.engine
                    nop.sync_info = SyncInfo(on_wait=[w], on_update=[])
                    insts.insert(idx, nop)
                si.on_wait = [waits[-1]]
                inst.sync_info = si


def get_nc() -> bass.Bass:
    if "nc" not in _NC_CACHE:
        nc = build_nc()
        _split_multi_waits(nc)
        _NC_CACHE["nc"] = nc
    return _NC_CACHE["nc"]


_ROW_IDX = np.concatenate([
    np.concatenate([np.arange(r, r + IMGH_ROWS), np.arange(r + 28, r + 28 + IMGH_ROWS)])
    for r in R0
])  # [4*60] padded-row indices per group (two 30-row halves, 2-row halo)


def make_in_maps(x: np.ndarray) -> list[dict]:
    x = np.asarray(x, dtype=np.float32)
    xp = np.pad(x, ((0, 0), (0, 0), (1, 1), (1, 1)))
    maps = []
    for core in range(N_CORES):
        b, half = divmod(core, 2)
        v = xp[b, half * CPC:(half + 1) * CPC]          # [32, 226, 226]
        v = v[:, _ROW_IDX, :].reshape(CPC, 4, 2 * IMGH_ROWS * PH)
        v = np.ascontiguousarray(v.transpose(1, 0, 2)).reshape(128, 2 * IMGH_F)
        maps.append({"xp": v})
    return maps


def gather_out(results: list[dict]) -> np.ndarray:
    out = np.empty((B, C * 9, OSZ), dtype=np.float32)
    for core in range(N_CORES):
        b, half = divmod(core, 2)
        out[b, half * NROW:(half + 1) * NROW] = results[core]["out"]
    return out


def kernel(**inputs) -> np.ndarray:
    x = inputs["x"]
    nc = get_nc()
    res = run_bass_kernel_spmd(nc, make_in_maps(x), list(range(N_CORES)))
    return gather_out(res.results)


# revision 9
# speedup vs baseline: 1.5292x; 1.0008x over previous
"""Unfold/im2col kernel for Trainium2 (Bass/Tile), 8-core data parallel.

Problem: x [4, 64, 224, 224] f32 -> out [4, 576, 49729] f32 where
out[b, (c*3+kh)*3+kw, oh*223+ow] = pad(x,1)[b, c, oh+kh, ow+kw]
(3x3 kernel, pad 1, stride 1, dilation 1, oh=ow=223).

Sharding: 8 cores = (batch 4) x (channel half 2). Each core handles
32 channels -> [288, 49729] independently; outputs concatenate on the
channel axis (channel-major row layout makes halves contiguous).

Per-core strategy (v4 — big-descriptor stores, half-window pipeline):
The input is zero-padded AND pre-sharded host-side into the exact
SBUF layout [128, 13560]: partition p = g*32 + c holds two 30-row
halves of padded rows of channel c (row-group g covers output rows
R0[g]..R0[g]+55, split into two 28-row halves with a 2-row halo
between the half tiles). Each of the two loads is then a full
128-partition DMA with one contiguous ~27 KB descriptor per
partition — keeping every SDMA engine on its own SBUF AXI port
(32-partition loads measured ~11 GB/s/engine from write-port
contention vs ~27 full-rate). For each of the 9 (kh, kw) windows,
DVE copies compact the 226-wide padded rows into the output's exact
DRAM layout (28 dense rows of 223) in half-window tiles (2 windows
double-buffered); each store DMA then moves ~25 KB fully-contiguous
runs per partition. This replaces the v1 pure-DMA scheme whose 892 B
descriptors bound the SDMA engines at ~230 GB/s; big descriptors run
at the HBM per-core roofline (~358 GB/s). Loads are issued on the ACT
HWDGE ring so their descriptors interleave with store descriptors
(SP ring) at the SDMA engines, letting the first stores overlap the
tail of the load phase; half-window granularity shortens the
load->copy->store serial head and the WAR wait for compacted-buffer
reuse. Group store order g0,g2,g1,g3 alternates the even-engine
(partitions 0-63) and odd-engine (64-127) SDMA halves.
"""

from contextlib import ExitStack

import numpy as np

import concourse.bass as bass
import concourse.tile as tile
from concourse import mybir
from concourse.ap import AP
from concourse.bass_utils import run_bass_kernel_spmd

B, C, IH, IW = 4, 64, 224, 224
N_CORES = 8
CPC = C // 2          # channels per core: 32
PH = IH + 2           # padded height/width: 226
OH = IH - 1           # output spatial: 223
OSZ = OH * OH         # 49729
NROW = CPC * 9        # 288 output rows per core
PIMG = PH * PH        # padded image elements: 51076

R0 = [0, 56, 112, 168]        # first output row of each group
IMGH_ROWS = 30                # padded rows per partition per half tile
IMGH_F = IMGH_ROWS * PH       # img half-tile free size: 6780
CBH_ROWS = 28                 # compacted rows per half (g3 h1 row 27 is junk)
CBH_F = CBH_ROWS * OH         # compacted half-tile free size: 6244

_NC_CACHE = {}


def build_nc() -> bass.Bass:
    nc = bass.Bass()
    x = nc.declare_dram_parameter("xp", [128, 2 * IMGH_F], mybir.dt.float32, isOutput=False)
    out = nc.declare_dram_parameter("out", [NROW, OSZ], mybir.dt.float32, isOutput=True)
    xb = x[:, :]
    ob = out[:, :]

    # interleave even-engine (g0,g1 -> partitions 0..63) and odd-engine
    # (g2,g3 -> partitions 64..127) groups so all 16 SDMA engines stay fed
    GORDER = (0, 2, 1, 3)

    with tile.TileContext(nc) as tc:
        with ExitStack() as ctx:
            pool = ctx.enter_context(tc.tile_pool(name="img", bufs=1))
            imgh = [
                pool.tile([128, IMGH_F], mybir.dt.float32, name=f"img{h}", tag=f"img{h}")[:, :]
                for h in range(2)
            ]
            # cb[j][h]: half h of window-buffer j (windows triple-buffered)
            cb = [
                [
                    pool.tile([128, CBH_F], mybir.dt.float32,
                              name=f"cb{j}{h}", tag=f"cb{j}{h}")[:, :]
                    for h in range(2)
                ]
                for j in range(3)
            ]

            # Loads (ACT HWDGE ring): the host pre-shards xp so that DRAM
            # row p is partition p's data; each load is one 128-partition
            # DMA with a 27120 B contiguous descriptor per partition.
            for h in range(2):
                src = AP(xb.tensor, xb.offset + h * IMGH_F,
                         [[2 * IMGH_F, 128], [1, IMGH_F]])
                dst = AP(imgh[h].tensor, imgh[h].offset,
                         [[IMGH_F, 128], [1, IMGH_F]])
                nc.scalar.dma_start(out=dst, in_=src)

            # Per window (kh, kw) and half h: DVE compacts 28 dense rows of
            # 223, then 4 store DMAs move ~25 KB contiguous runs. Stores
            # alternate between the SP and ACT HWDGE rings per window so a
            # copy-sem wait at one ring's head can't starve the SDMA
            # engines — the other ring's queued descriptors keep flowing.
            for w in range(9):
                kh, kw = divmod(w, 3)
                st_eng = nc.sync if w % 2 == 0 else nc.scalar
                for h in range(2):
                    buf = cb[w % 3][h]
                    c_src = AP(imgh[h].tensor, imgh[h].offset + kh * PH + kw,
                               [[IMGH_F, 128], [PH, CBH_ROWS], [1, OH]])
                    c_dst = AP(buf.tensor, buf.offset,
                               [[CBH_F, 128], [OH, CBH_ROWS], [1, OH]])
                    nc.vector.tensor_copy(out=c_dst, in_=c_src)

                    for g in GORDER:
                        n = 27 if (g == 3 and h == 1) else 28
                        s = AP(buf.tensor,
                               buf.offset + (g * CPC) * CBH_F,
                               [[CBH_F, CPC], [1, n * OH]])
                        d = AP(ob.tensor,
                               ob.offset + (kh * 3 + kw) * OSZ + (R0[g] + 28 * h) * OH,
                               [[9 * OSZ, CPC], [1, n * OH]])
                        st_eng.dma_start(out=d, in_=s)
    return nc


def _split_multi_waits(nc: bass.Bass) -> None:
    """Walrus allows only one sync-wait command per instruction (the
    kernel-tail drain ends up with one per DMA-completion sem lane).
    Hoist all but the last wait onto fresh single-wait NOPs inserted
    just before the instruction on the same engine — semantically
    identical (the engine blocks on each wait in turn)."""
    from bass_rust import SyncInfo

    k = 0
    for fn in nc.m.functions:
        for blk in fn.blocks:
            insts = blk.instructions
            for idx in range(len(insts) - 1, -1, -1):
                inst = insts[idx]
                si = inst.sync_info
                if si is None or len(si.on_wait) <= 1:
                    continue
                waits = list(si.on_wait)
                for w in waits[:-1]:
                    nop = mybir.InstNoOp(name=f"WSPLIT-{k}")
                    k += 1
                    nop.engine = inst.engine
                    nop.sync_info = SyncInfo(on_wait=[w], on_update=[])
                    insts.insert(idx, nop)
                si.on_wait = [waits[-1]]
                inst.sync_info = si


def get_nc() -> bass.Bass:
    if "nc" not in _NC_CACHE:
        nc = build_nc()
        _split_multi_waits(nc)
        _NC_CACHE["nc"] = nc
    return _NC_CACHE["nc"]


_ROW_IDX = np.concatenate([
    np.concatenate([np.arange(r, r + IMGH_ROWS), np.arange(r + 28, r + 28 + IMGH_ROWS)])
    for r in R0
])  # [4*60] padded-row indices per group (two 30-row halves, 2-row halo)


def make_in_maps(x: np.ndarray) -> list[dict]:
    x = np.asarray(x, dtype=np.float32)
    xp = np.pad(x, ((0, 0), (0, 0), (1, 1), (1, 1)))
    maps = []
    for core in range(N_CORES):
        b, half = divmod(core, 2)
        v = xp[b, half * CPC:(half + 1) * CPC]          # [32, 226, 226]
        v = v[:, _ROW_IDX, :].reshape(CPC, 4, 2 * IMGH_ROWS * PH)
        v = np.ascontiguousarray(v.transpose(1, 0, 2)).reshape(128, 2 * IMGH_F)
        maps.append({"xp": v})
    return maps


def gather_out(results: list[dict]) -> np.ndarray:
    out = np.empty((B, C * 9, OSZ), dtype=np.float32)
    for core in range(N_CORES):
        b, half = divmod(core, 2)
        out[b, half * NROW:(half + 1) * NROW] = results[core]["out"]
    return out


def kernel(**inputs) -> np.ndarray:
    x = inputs["x"]
    nc = get_nc()
    res = run_bass_kernel_spmd(nc, make_in_maps(x), list(range(N_CORES)))
    return gather_out(res.results)
